# revision 15
# baseline (speedup 1.0000x reference)
"""Trainium2 Bass kernel for the LayerNorm-RNN attention variant.

Math (per batch element b, reference semantics):
    u_t   = (x_t @ W_e2s + b_e2s) @ Bm                      # injected input
    y_t   = s_{t-1} @ A + u_t
    s_t   = LN(y_t) * gamma + beta                          # LayerNorm over S
    out_t = (s_t @ C) @ W_s2o + b_s2o

Host-side folds (exact linear algebra, done once in fp32/fp64):
    P    = I - (1/S) 11^T              centering projector
    W_u  = (W_e2s @ Bm) @ P            u~_t = x_t @ W_u + (b_e2s@Bm + beta@A)@P
    G~   = (diag(gamma) @ A) @ P       (bf16)
    W_o  = (diag(gamma) @ C) @ W_s2o   b_out = beta @ C @ W_s2o + b_s2o

Scan recurrence in CENTERED coordinates: track zc_t = y_t - mean(y_t).
Since LN is shift invariant and (z - mu) @ G = zc @ G, centering G and u
once on the host removes all mean handling from the device loop:
    zc_{t+1} = rr_t * (zc_t @ G~) + u~_{t+1},   rr_t = rsqrt(var_t + eps)
    var_t    = sum(zc_t^2) / S                  (zc is exactly mean-free)
    s_t      = rr_t * zc_t                      (whitened state; gamma/beta
                                                 folded into the output GEMM)
Per-step critical chain (one op per engine hop):
    STT (DVE)  zc_{t+1} = rr*wp + u~          [wp read straight from PSUM]
    TTR (DVE)  zsq = zc*zc, accum ss = per-partition sums   [fused reduce]
    MM  (PE)   var = (1/S) ones^T ss          [cross-partition + broadcast]
    ACT        rr  = rsqrt(var + eps)

Sharding: data-parallel over batch, 1 batch element per NeuronCore (8 cores).
Layouts on chip are column-form: S=512 lives as [128 partitions x 4 free].
"""

import sys
import os
from contextlib import ExitStack

import numpy as np

for _p in ("/opt/trn_rl_repo",):
    if _p not in sys.path and os.path.isdir(_p):
        sys.path.insert(0, _p)

B, T, E, S = 8, 2048, 1024, 512
LN_EPS = 1e-5
NCORES = 8
UNROLL = 256
VARIANT = os.environ.get("KVARIANT", "")

_CACHE = {}


def build(t_len=T, unroll=UNROLL):
    """Build the single-core Bass program (SPMD across 8 cores)."""
    import concourse.bass as bass
    import concourse.bacc as bacc
    from concourse import mybir
    from concourse.tile import TileContext
    from concourse.tile_rust import add_dep_helper

    f32 = mybir.dt.float32
    bf16 = mybir.dt.bfloat16
    AF = mybir.ActivationFunctionType
    ALU = mybir.AluOpType
    ds = bass.ds

    n_iters = t_len // unroll
    assert n_iters * unroll == t_len and unroll % 2 == 0
    n_tc = t_len // 512 if t_len >= 512 else 1   # pre-pass t-chunks
    tcw = min(512, t_len)                        # pre-pass chunk width
    pcw = min(128, t_len)                        # post-pass chunk width
    n_pc = (t_len + pcw - 1) // pcw              # post-pass t-chunks

    nc = bacc.Bacc(trn_type="TRN2")

    xt = nc.dram_tensor("xt", [E, t_len], bf16, kind="ExternalInput")
    wu = nc.dram_tensor("wu", [8, 4, 128, 128], bf16, kind="ExternalInput")
    gt = nc.dram_tensor("gt", [4, 4, 128, 128], bf16, kind="ExternalInput")
    wo = nc.dram_tensor("wo", [S, E], bf16, kind="ExternalInput")
    bud = nc.dram_tensor("buc", [128, 4], f32, kind="ExternalInput")
    bod = nc.dram_tensor("bo", [1, E], f32, kind="ExternalInput")
    cnegd = nc.dram_tensor("cneg", [128, 4], f32, kind="ExternalInput")
    onesd = nc.dram_tensor("ones", [128, 128], bf16, kind="ExternalInput")
    y = nc.dram_tensor("y", [t_len, E], f32, kind="ExternalOutput")

    with ExitStack() as ctx:
        tc = ctx.enter_context(TileContext(nc))
        singles = ctx.enter_context(tc.tile_pool(name="singles", bufs=1))
        xpool = ctx.enter_context(tc.tile_pool(name="xpool", bufs=16))
        psum_big = ctx.enter_context(tc.tile_pool(name="psum_big", bufs=2, space="PSUM"))
        psum_w = ctx.enter_context(tc.tile_pool(name="psum_w", bufs=3, space="PSUM"))
        psum_ab = ctx.enter_context(tc.tile_pool(name="psum_ab", bufs=3, space="PSUM"))
        opool = ctx.enter_context(tc.tile_pool(name="opool", bufs=2))

        # ---- resident weights / constants ----
        wu_sb = singles.tile([128, 8, 4, 128], bf16)
        nc.sync.dma_start(out=wu_sb, in_=wu.rearrange("k m p q -> p k m q"))
        gt_sb = singles.tile([128, 4, 4, 128], bf16)
        nc.sync.dma_start(out=gt_sb, in_=gt.rearrange("k m p q -> p k m q"))
        wo_sb = singles.tile([128, 4, E], bf16)
        nc.sync.dma_start(out=wo_sb, in_=wo.rearrange("(k p) e -> p k e", p=128))
        bu_sb = singles.tile([128, 4], f32)
        nc.sync.dma_start(out=bu_sb, in_=bud[:])
        ones_sb = singles.tile([128, 128], bf16)
        nc.sync.dma_start(out=ones_sb, in_=onesd[:])
        bo_ap = bod[:]
        bo_sb = singles.tile([128, E], f32)
        nc.sync.dma_start(
            out=bo_sb,
            in_=bass.AP(tensor=bo_ap.tensor, offset=bo_ap.offset, ap=[[0, 128], [1, E]]),
        )
        cneg_sb = singles.tile([128, 4], f32)
        nc.sync.dma_start(out=cneg_sb, in_=cnegd[:])
        eps_sb = singles.tile([128, 1], f32)
        nc.vector.memset(eps_sb, LN_EPS)

        u_col = singles.tile([128, (t_len + 1) * 4], f32)
        states = singles.tile([128, t_len * 4], bf16)
        st_view = states.rearrange("p (t f) -> p t f", f=4)
        u_view = u_col.rearrange("p (t f) -> p t f", f=4)
        nc.vector.memset(u_col[:, t_len * 4:(t_len + 1) * 4], 0.0)

        # ---- pre-pass: u~_col[s, t] = (x @ W_u + b_u).T in column form ----
        for c in range(n_tc):
            xts = []
            for e in range(8):
                xt_t = xpool.tile([128, tcw], bf16, tag="xt")
                nc.sync.dma_start(
                    out=xt_t, in_=xt[e * 128:(e + 1) * 128, c * tcw:(c + 1) * tcw]
                )
                xts.append(xt_t)
            for m in range(4):
                ps = psum_big.tile([128, tcw], f32)
                for k in range(8):
                    nc.tensor.matmul(
                        ps, wu_sb[:, k, m, :], xts[k], start=(k == 0), stop=(k == 7)
                    )
                nc.scalar.activation(
                    out=u_view[:, c * tcw:(c + 1) * tcw, m],
                    in_=ps,
                    func=AF.Identity,
                    bias=bu_sb[:, m:m + 1],
                    scale=1.0,
                )

        # ---- scan ----
        z_a = singles.tile([128, 8], bf16)
        z_b = singles.tile([128, 8], bf16)
        rbs = [singles.tile([128, 1], f32, name=f"rb{i}") for i in range(4)]
        sss = [singles.tile([128, 1], bf16, name=f"ss{i}") for i in range(4)]
        # staging tiles: one dynamic DMA per loop iteration instead of one
        # register-offset AP per step (engines run out of registers past ~6)
        u_stg = singles.tile([128, (unroll + 1) * 4], f32)
        st_stg = singles.tile([128, unroll * 4], bf16)

        # prologue: zc_0 = u~_0 - (beta@A)@P (state at t=-1 is exactly zero, so
        # the beta-fold baked into b_u must be removed for step 0)
        nc.vector.tensor_add(z_a[:, 0:4], u_col[:, 0:4], cneg_sb)

        def scan_step(jj):
            even = jj % 2 == 0
            cur_z, nxt_z = (z_a, z_b) if even else (z_b, z_a)
            rb = rbs[jj % 4]
            ss = sss[jj % 4]
            zc = cur_z[:, 0:4]
            zsq = cur_z[:, 4:8]
            # zsq = zc*zc AND ss = per-partition sum (one fused custom DVE op;
            # bf16 ss = one rounding of the partial sums, feeds a bf16 MM)
            with nc.allow_low_precision("variance partials, single rounding"):
                nc.vector.affine_mul_reduce(
                    out=zsq, accum_out=ss, in0=zc, in1=zc, scale=1.0, bias=0.0,
                )
            # W = zc @ G~ (PE, 16 [128,128] bf16 tiles) with the variance
            # broadcast MM pinned at position 11 of the PE stream: 10 W pairs
            # run while the fused square produces ss, then the stats MM fires
            # as soon as ss lands, then the remaining 6 W pairs. This balances
            # the rsqrt path and the wp path at the downstream STT.
            vv = psum_ab.tile([128, 1], f32)
            wp = psum_w.tile([128, 4], f32)
            w_mms = []
            for m in range(4):
                for kk in range(4):
                    w_mms.append(nc.tensor.matmul(
                        wp[:, m:m + 1], gt_sb[:, kk, m, :], zc[:, kk:kk + 1],
                        start=(kk == 0), stop=(kk == 3),
                        skip_group_check=True,
                    ))
            mm_stats = nc.tensor.matmul(vv, ones_sb, ss, start=True, stop=True,
                                        skip_group_check=True)
            add_dep_helper(mm_stats.ins, w_mms[9].ins, sync=False,
                           reason="stats MM after first 10 W pairs")
            add_dep_helper(w_mms[10].ins, mm_stats.ins, sync=False,
                           reason="last 6 W pairs after stats MM")
            # rr = 1/sqrt(var + eps), reading PSUM directly
            nc.scalar.activation(
                out=rb, in_=vv, func=AF.Abs_reciprocal_sqrt,
                bias=eps_sb, scale=1.0,
            )
            # whitened state rr*zc -> states buffer (ACT, right after the
            # rsqrt in FIFO order: the next STT's WAR on this z tile then
            # collapses into its existing rb wait — no extra semaphore)
            nc.scalar.activation(
                out=st_stg[:, jj * 4:(jj + 1) * 4], in_=zc, func=AF.Identity,
                scale=rb,
            )
            # serial tail: zc_{k+1} = rr*wp + u~[k+1]
            un = u_stg[:, (jj + 1) * 4:(jj + 2) * 4]
            nc.vector.scalar_tensor_tensor(
                out=nxt_z[:, 0:4], in0=wp, scalar=rb, in1=un,
                op0=ALU.mult, op1=ALU.add,
            )

        ucw = unroll * 4
        with tc.For_i(0, n_iters, 1, hint_engines=(
                mybir.EngineType.PE, mybir.EngineType.DVE,
                mybir.EngineType.Activation)) as iv:
            # stage u~[k] for this slab (shifted +1 step: the STT of step jj
            # reads u~[iv*unroll+jj+1]); the states flush of the previous slab
            # rides the sync queue so the two boundary DMAs run in parallel
            nc.gpsimd.dma_start(out=u_stg, in_=u_col[:, ds(iv * ucw, ucw + 4)])
            for jj in range(unroll):
                scan_step(jj)
            nc.sync.dma_start(out=states[:, ds(iv * ucw, ucw)], in_=st_stg)

        # ---- post-pass: out = states @ W_o + b_out ----
        for t_i in range(n_pc):
            ob = opool.tile([128, E], f32)
            for ec in range(2):
                ps = psum_big.tile([128, 512], f32)
                for kk in range(4):
                    nc.tensor.matmul(
                        ps[:pcw, :],
                        st_view[:, t_i * pcw:(t_i + 1) * pcw, kk],
                        wo_sb[:, kk, ec * 512:(ec + 1) * 512],
                        start=(kk == 0),
                        stop=(kk == 3),
                    )
                nc.vector.tensor_add(
                    ob[:pcw, ec * 512:(ec + 1) * 512], ps[:pcw, :],
                    bo_sb[:pcw, ec * 512:(ec + 1) * 512]
                )
            nc.sync.dma_start(out=y[t_i * pcw:(t_i + 1) * pcw, :], in_=ob[:pcw, :])

    nc.compile()
    return nc


def host_prep(inputs, t_len=T):
    """Fold parameters on the host; returns (shared dict, per-core xt list)."""
    from ml_dtypes import bfloat16

    et = np.asarray(inputs["embedded_tokens"], np.float32)
    W_e2s = np.asarray(inputs["W_e2s"], np.float64)
    b_e2s = np.asarray(inputs["b_e2s"], np.float64)
    A = np.asarray(inputs["A"], np.float64)
    Bm = np.asarray(inputs["Bm"], np.float64)
    C = np.asarray(inputs["C"], np.float64)
    gamma = np.asarray(inputs["ln_gamma"], np.float64)
    beta = np.asarray(inputs["ln_beta"], np.float64)
    W_s2o = np.asarray(inputs["W_s2o"], np.float64)
    b_s2o = np.asarray(inputs["b_s2o"], np.float64)

    P = np.eye(S) - 1.0 / S                                    # centering
    W_u = ((W_e2s @ Bm) @ P).astype(np.float32)                # [E, S]
    b_u = ((b_e2s @ Bm + beta @ A) @ P).astype(np.float32)     # [S]
    G = ((gamma[:, None] * A) @ P).astype(np.float32)          # [S, S]
    Gb = G.astype(bfloat16)
    W_o = ((gamma[:, None] * C) @ W_s2o).astype(np.float32)    # [S, E]
    b_out = (beta @ C @ W_s2o + b_s2o).astype(np.float32)      # [E]

    wu_tiles = np.ascontiguousarray(
        W_u.reshape(8, 128, 4, 128).transpose(0, 2, 1, 3)
    )  # [k, m, 128, 128]
    gt_tiles = np.ascontiguousarray(
        Gb.reshape(4, 128, 4, 128).transpose(0, 2, 1, 3)
    )  # [k, m, 128, 128] bf16

    shared = {
        "wu": wu_tiles.astype(bfloat16),
        "gt": gt_tiles,
        "wo": np.ascontiguousarray(W_o.astype(bfloat16)),
        "buc": np.ascontiguousarray(b_u.reshape(4, 128).T),
        "bo": np.ascontiguousarray(b_out.reshape(1, E)),
        "cneg": np.ascontiguousarray(
            (-((beta @ A) @ P)).astype(np.float32).reshape(4, 128).T
        ),
        "ones": np.full((128, 128), 1.0 / S, bfloat16),
    }
    xts = [
        np.ascontiguousarray(et[b, :t_len, :].T.astype(bfloat16))
        for b in range(et.shape[0])
    ]
    return shared, xts


def kernel(**inputs):
    key = ("nc", T, UNROLL)
    if key not in _CACHE:
        _CACHE[key] = build(T, UNROLL)
    nc = _CACHE[key]

    from concourse.bass_utils import run_bass_kernel_spmd

    shared, xts = host_prep(inputs)
    in_maps = [dict(shared, xt=xts[b]) for b in range(B)]
    res = run_bass_kernel_spmd(nc, in_maps, core_ids=list(range(NCORES)))
    out = np.stack([np.asarray(r["y"], np.float32) for r in res.results], axis=0)
    return out


# revision 16
# speedup vs baseline: 1.1933x; 1.1933x over previous
"""Trainium2 Bass kernel for the LayerNorm-RNN attention variant.

Math (per batch element b, reference semantics):
    u_t   = (x_t @ W_e2s + b_e2s) @ Bm                      # injected input
    y_t   = s_{t-1} @ A + u_t
    s_t   = LN(y_t) * gamma + beta                          # LayerNorm over S
    out_t = (s_t @ C) @ W_s2o + b_s2o

Host-side folds (exact linear algebra, done once in fp32/fp64):
    P    = I - (1/S) 11^T              centering projector
    W_u  = (W_e2s @ Bm) @ P            u~_t = x_t @ W_u + (b_e2s@Bm + beta@A)@P
    G~   = (diag(gamma) @ A) @ P       (bf16)
    W_o  = (diag(gamma) @ C) @ W_s2o   b_out = beta @ C @ W_s2o + b_s2o

Scan recurrence in CENTERED coordinates: track zc_t = y_t - mean(y_t).
Since LN is shift invariant and (z - mu) @ G = zc @ G, centering G and u
once on the host removes all mean handling from the device loop:
    zc_{t+1} = rr_t * (zc_t @ G~) + u~_{t+1},   rr_t = rsqrt(var_t + eps)
    var_t    = sum(zc_t^2) / S                  (zc is exactly mean-free)
    s_t      = rr_t * zc_t                      (whitened state; gamma/beta
                                                 folded into the output GEMM)
Per-step critical chain (one op per engine hop):
    STT (DVE)  zc_{t+1} = rr*wp + u~          [wp read straight from PSUM]
    TTR (DVE)  zsq = zc*zc, accum ss = per-partition sums   [fused reduce]
    MM  (PE)   var = (1/S) ones^T ss          [cross-partition + broadcast]
    ACT        rr  = rsqrt(var + eps)

Sharding: data-parallel over batch, 1 batch element per NeuronCore (8 cores).
Layouts on chip are column-form: S=512 lives as [128 partitions x 4 free].
"""

import sys
import os
from contextlib import ExitStack

import numpy as np

for _p in ("/opt/trn_rl_repo",):
    if _p not in sys.path and os.path.isdir(_p):
        sys.path.insert(0, _p)

B, T, E, S = 8, 2048, 1024, 512
LN_EPS = 1e-5
NCORES = 8
UNROLL = 256
VARIANT = os.environ.get("KVARIANT", "")

_CACHE = {}


def build(t_len=T, unroll=UNROLL):
    """Build the single-core Bass program (SPMD across 8 cores)."""
    import concourse.bass as bass
    import concourse.bacc as bacc
    from concourse import mybir
    from concourse.tile import TileContext
    from concourse.tile_rust import add_dep_helper

    f32 = mybir.dt.float32
    bf16 = mybir.dt.bfloat16
    AF = mybir.ActivationFunctionType
    ALU = mybir.AluOpType
    ds = bass.ds

    n_iters = t_len // unroll
    assert n_iters * unroll == t_len and unroll % 2 == 0
    n_tc = t_len // 512 if t_len >= 512 else 1   # pre-pass t-chunks
    tcw = min(512, t_len)                        # pre-pass chunk width
    pcw = min(128, t_len)                        # post-pass chunk width
    n_pc = (t_len + pcw - 1) // pcw              # post-pass t-chunks

    nc = bacc.Bacc(trn_type="TRN2")

    xt = nc.dram_tensor("xt", [E, t_len], bf16, kind="ExternalInput")
    wu = nc.dram_tensor("wu", [8, 4, 128, 128], bf16, kind="ExternalInput")
    gt = nc.dram_tensor("gt", [4, 4, 128, 128], bf16, kind="ExternalInput")
    wo = nc.dram_tensor("wo", [S, E], bf16, kind="ExternalInput")
    bud = nc.dram_tensor("buc", [128, 4], f32, kind="ExternalInput")
    bod = nc.dram_tensor("bo", [1, E], f32, kind="ExternalInput")
    cnegd = nc.dram_tensor("cneg", [128, 4], f32, kind="ExternalInput")
    onesd = nc.dram_tensor("ones", [128, 128], bf16, kind="ExternalInput")
    y = nc.dram_tensor("y", [t_len, E], f32, kind="ExternalOutput")

    with ExitStack() as ctx:
        tc = ctx.enter_context(TileContext(nc))
        singles = ctx.enter_context(tc.tile_pool(name="singles", bufs=1))
        xpool = ctx.enter_context(tc.tile_pool(name="xpool", bufs=16))
        psum_big = ctx.enter_context(tc.tile_pool(name="psum_big", bufs=2, space="PSUM"))
        psum_w = ctx.enter_context(tc.tile_pool(name="psum_w", bufs=3, space="PSUM"))
        psum_ab = ctx.enter_context(tc.tile_pool(name="psum_ab", bufs=3, space="PSUM"))
        opool = ctx.enter_context(tc.tile_pool(name="opool", bufs=2))

        # ---- resident weights / constants ----
        wu_sb = singles.tile([128, 8, 4, 128], bf16)
        nc.sync.dma_start(out=wu_sb, in_=wu.rearrange("k m p q -> p k m q"))
        gt_sb = singles.tile([128, 4, 4, 128], bf16)
        nc.sync.dma_start(out=gt_sb, in_=gt.rearrange("k m p q -> p k m q"))
        wo_sb = singles.tile([128, 4, E], bf16)
        nc.sync.dma_start(out=wo_sb, in_=wo.rearrange("(k p) e -> p k e", p=128))
        bu_sb = singles.tile([128, 4], f32)
        nc.sync.dma_start(out=bu_sb, in_=bud[:])
        ones_sb = singles.tile([128, 128], bf16)
        nc.sync.dma_start(out=ones_sb, in_=onesd[:])
        bo_ap = bod[:]
        bo_sb = singles.tile([128, E], f32)
        nc.sync.dma_start(
            out=bo_sb,
            in_=bass.AP(tensor=bo_ap.tensor, offset=bo_ap.offset, ap=[[0, 128], [1, E]]),
        )
        cneg_sb = singles.tile([128, 4], f32)
        nc.sync.dma_start(out=cneg_sb, in_=cnegd[:])
        eps_sb = singles.tile([128, 1], f32)
        nc.vector.memset(eps_sb, LN_EPS)

        u_col = singles.tile([128, (t_len + 1) * 4], f32)
        states = singles.tile([128, t_len * 4], bf16)
        st_view = states.rearrange("p (t f) -> p t f", f=4)
        u_view = u_col.rearrange("p (t f) -> p t f", f=4)
        nc.vector.memset(u_col[:, t_len * 4:(t_len + 1) * 4], 0.0)

        # ---- pre-pass: u~_col[s, t] = (x @ W_u + b_u).T in column form ----
        for c in range(n_tc):
            xts = []
            for e in range(8):
                xt_t = xpool.tile([128, tcw], bf16, tag="xt")
                nc.sync.dma_start(
                    out=xt_t, in_=xt[e * 128:(e + 1) * 128, c * tcw:(c + 1) * tcw]
                )
                xts.append(xt_t)
            for m in range(4):
                ps = psum_big.tile([128, tcw], f32)
                for k in range(8):
                    nc.tensor.matmul(
                        ps, wu_sb[:, k, m, :], xts[k], start=(k == 0), stop=(k == 7)
                    )
                nc.scalar.activation(
                    out=u_view[:, c * tcw:(c + 1) * tcw, m],
                    in_=ps,
                    func=AF.Identity,
                    bias=bu_sb[:, m:m + 1],
                    scale=1.0,
                )

        # ---- scan ----
        z_a = singles.tile([128, 8], bf16)
        z_b = singles.tile([128, 8], bf16)
        rbs = [singles.tile([128, 1], f32, name=f"rb{i}") for i in range(4)]
        sss = [singles.tile([128, 1], bf16, name=f"ss{i}") for i in range(4)]
        # staging tiles: one dynamic DMA per loop iteration instead of one
        # register-offset AP per step (engines run out of registers past ~6)
        u_stg = singles.tile([128, (unroll + 1) * 4], f32)
        st_stg = singles.tile([128, unroll * 4], bf16)

        # prologue: zc_0 = u~_0 - (beta@A)@P (state at t=-1 is exactly zero, so
        # the beta-fold baked into b_u must be removed for step 0)
        nc.vector.tensor_add(z_a[:, 0:4], u_col[:, 0:4], cneg_sb)

        def scan_step(jj):
            even = jj % 2 == 0
            cur_z, nxt_z = (z_a, z_b) if even else (z_b, z_a)
            rb = rbs[jj % 4]
            ss = sss[jj % 4]
            zc = cur_z[:, 0:4]
            zsq = cur_z[:, 4:8]
            # zsq = zc*zc AND ss = per-partition sum (one fused custom DVE op;
            # bf16 ss = one rounding of the partial sums, feeds a bf16 MM)
            with nc.allow_low_precision("variance partials, single rounding"):
                nc.vector.affine_mul_reduce(
                    out=zsq, accum_out=ss, in0=zc, in1=zc, scale=1.0, bias=0.0,
                )
            # W = zc @ G~ (PE, 16 [128,128] bf16 tiles) with the variance
            # broadcast MM pinned at position 11 of the PE stream: 10 W pairs
            # run while the fused square produces ss, then the stats MM fires
            # as soon as ss lands, then the remaining 6 W pairs. This balances
            # the rsqrt path and the wp path at the downstream STT.
            vv = psum_ab.tile([128, 1], f32)
            wp = psum_w.tile([128, 4], f32)
            w_mms = []
            for m in range(4):
                for kk in range(4):
                    w_mms.append(nc.tensor.matmul(
                        wp[:, m:m + 1], gt_sb[:, kk, m, :], zc[:, kk:kk + 1],
                        start=(kk == 0), stop=(kk == 3),
                        skip_group_check=True,
                    ))
            mm_stats = nc.tensor.matmul(vv, ones_sb, ss, start=True, stop=True,
                                        skip_group_check=True)
            add_dep_helper(mm_stats.ins, w_mms[9].ins, sync=False,
                           reason="stats MM after first 10 W pairs")
            add_dep_helper(w_mms[10].ins, mm_stats.ins, sync=False,
                           reason="last 6 W pairs after stats MM")
            # rr = 1/sqrt(var + eps), reading PSUM directly
            nc.scalar.activation(
                out=rb, in_=vv, func=AF.Abs_reciprocal_sqrt,
                bias=eps_sb, scale=1.0,
            )
            # whitened state rr*zc -> states buffer (ACT, right after the
            # rsqrt in FIFO order: the next STT's WAR on this z tile then
            # collapses into its existing rb wait — no extra semaphore)
            nc.scalar.activation(
                out=st_stg[:, jj * 4:(jj + 1) * 4], in_=zc, func=AF.Identity,
                scale=rb,
            )
            # serial tail: zc_{k+1} = rr*wp + u~[k+1]
            un = u_stg[:, (jj + 1) * 4:(jj + 2) * 4]
            nc.vector.scalar_tensor_tensor(
                out=nxt_z[:, 0:4], in0=wp, scalar=rb, in1=un,
                op0=ALU.mult, op1=ALU.add,
            )

        ucw = unroll * 4
        with tc.For_i(0, n_iters, 1, hint_engines=(
                mybir.EngineType.PE, mybir.EngineType.DVE,
                mybir.EngineType.Activation)) as iv:
            # stage u~[k] for this slab (shifted +1 step: the STT of step jj
            # reads u~[iv*unroll+jj+1])
            nc.gpsimd.dma_start(out=u_stg, in_=u_col[:, ds(iv * ucw, ucw + 4)])
            for jj in range(unroll):
                scan_step(jj)
            # flush whitened states for this slab
            nc.gpsimd.dma_start(out=states[:, ds(iv * ucw, ucw)], in_=st_stg)

        # ---- post-pass: out = states @ W_o + b_out ----
        for t_i in range(n_pc):
            ob = opool.tile([128, E], f32)
            for ec in range(2):
                ps = psum_big.tile([128, 512], f32)
                for kk in range(4):
                    nc.tensor.matmul(
                        ps[:pcw, :],
                        st_view[:, t_i * pcw:(t_i + 1) * pcw, kk],
                        wo_sb[:, kk, ec * 512:(ec + 1) * 512],
                        start=(kk == 0),
                        stop=(kk == 3),
                    )
                nc.vector.tensor_add(
                    ob[:pcw, ec * 512:(ec + 1) * 512], ps[:pcw, :],
                    bo_sb[:pcw, ec * 512:(ec + 1) * 512]
                )
            nc.sync.dma_start(out=y[t_i * pcw:(t_i + 1) * pcw, :], in_=ob[:pcw, :])

    nc.compile()
    return nc


def host_prep(inputs, t_len=T):
    """Fold parameters on the host; returns (shared dict, per-core xt list)."""
    from ml_dtypes import bfloat16

    et = np.asarray(inputs["embedded_tokens"], np.float32)
    W_e2s = np.asarray(inputs["W_e2s"], np.float64)
    b_e2s = np.asarray(inputs["b_e2s"], np.float64)
    A = np.asarray(inputs["A"], np.float64)
    Bm = np.asarray(inputs["Bm"], np.float64)
    C = np.asarray(inputs["C"], np.float64)
    gamma = np.asarray(inputs["ln_gamma"], np.float64)
    beta = np.asarray(inputs["ln_beta"], np.float64)
    W_s2o = np.asarray(inputs["W_s2o"], np.float64)
    b_s2o = np.asarray(inputs["b_s2o"], np.float64)

    P = np.eye(S) - 1.0 / S                                    # centering
    W_u = ((W_e2s @ Bm) @ P).astype(np.float32)                # [E, S]
    b_u = ((b_e2s @ Bm + beta @ A) @ P).astype(np.float32)     # [S]
    G = ((gamma[:, None] * A) @ P).astype(np.float32)          # [S, S]
    Gb = G.astype(bfloat16)
    W_o = ((gamma[:, None] * C) @ W_s2o).astype(np.float32)    # [S, E]
    b_out = (beta @ C @ W_s2o + b_s2o).astype(np.float32)      # [E]

    wu_tiles = np.ascontiguousarray(
        W_u.reshape(8, 128, 4, 128).transpose(0, 2, 1, 3)
    )  # [k, m, 128, 128]
    gt_tiles = np.ascontiguousarray(
        Gb.reshape(4, 128, 4, 128).transpose(0, 2, 1, 3)
    )  # [k, m, 128, 128] bf16

    shared = {
        "wu": wu_tiles.astype(bfloat16),
        "gt": gt_tiles,
        "wo": np.ascontiguousarray(W_o.astype(bfloat16)),
        "buc": np.ascontiguousarray(b_u.reshape(4, 128).T),
        "bo": np.ascontiguousarray(b_out.reshape(1, E)),
        "cneg": np.ascontiguousarray(
            (-((beta @ A) @ P)).astype(np.float32).reshape(4, 128).T
        ),
        "ones": np.full((128, 128), 1.0 / S, bfloat16),
    }
    xts = [
        np.ascontiguousarray(et[b, :t_len, :].T.astype(bfloat16))
        for b in range(et.shape[0])
    ]
    return shared, xts


def kernel(**inputs):
    key = ("nc", T, UNROLL)
    if key not in _CACHE:
        _CACHE[key] = build(T, UNROLL)
    nc = _CACHE[key]

    from concourse.bass_utils import run_bass_kernel_spmd

    shared, xts = host_prep(inputs)
    in_maps = [dict(shared, xt=xts[b]) for b in range(B)]
    res = run_bass_kernel_spmd(nc, in_maps, core_ids=list(range(NCORES)))
    out = np.stack([np.asarray(r["y"], np.float32) for r in res.results], axis=0)
    return out
